# revision 6
# baseline (speedup 1.0000x reference)
"""Mamba (selective SSM) layer on 8 Trainium2 NeuronCores via Bass/Tile.

Sharding: tensor-parallel over d_inner (512 channels/core). x_proj partial
sums AllReduced (2050x96) so every core sees full dt_in/B/C. Scan layout:
[d on partitions, time on free], DVE tensor_tensor_scan per (d-tile, n),
A folded into ACT exp scale (A rows constant across d for S4D init; general
fallback uses per-partition scale vectors). Host sums out_proj partials.
"""
import numpy as np
import ml_dtypes

import concourse.bass as bass
import concourse.bacc as bacc
import concourse.tile as tile
from concourse import mybir
from concourse.bass_utils import run_bass_kernel_spmd

F32 = mybir.dt.float32
F32R = mybir.dt.float32r
BF16 = mybir.dt.bfloat16
AF = mybir.ActivationFunctionType
OP = mybir.AluOpType

B = 2
L = 1025            # seq len incl. prepended emb token
TT = B * L          # 2050 tokens, b-major
DM = 1024
DI = 4096
N_CORES = 8
DLOC = DI // N_CORES  # 512
NDT = DLOC // 128     # 4 d-tiles
NST = 16
DTR = 64
XD = DTR + 2 * NST    # 96

TCH = [512, 512, 512, 512, 2]
TCHO = [0, 512, 1024, 1536, 2048]


def _tslices():
    out, t = [], 0
    while t < TT:
        w = min(128, TT - t)
        out.append((t, w))
        t += w
    return out


def build_program(a_scales, debug=False):
    nc = bacc.Bacc(trn_type="TRN2")

    xsT = nc.dram_tensor("xsT", [DM, TT], F32R, kind="ExternalInput")
    w_in = nc.dram_tensor("w_in", [DM, 2 * DLOC], F32R, kind="ExternalInput")
    conv_w = nc.dram_tensor("conv_w", [DLOC, 4], F32, kind="ExternalInput")
    conv_b = nc.dram_tensor("conv_b", [DLOC, 1], F32, kind="ExternalInput")
    w_x = nc.dram_tensor("w_x", [DLOC, XD], BF16, kind="ExternalInput")
    w_dt = nc.dram_tensor("w_dt", [DTR, DLOC], BF16, kind="ExternalInput")
    b_dt = nc.dram_tensor("b_dt", [DLOC, 1], F32, kind="ExternalInput")
    a_cols = nc.dram_tensor("a_cols", [DLOC, NST], F32, kind="ExternalInput")
    d_skip = nc.dram_tensor("d_skip", [DLOC, 1], F32, kind="ExternalInput")
    w_out = nc.dram_tensor("w_out", [DLOC, DM], BF16, kind="ExternalInput")
    ident = nc.dram_tensor("ident", [128, 128], F32, kind="ExternalInput")
    out_p = nc.dram_tensor("out_p", [TT, DM], F32, kind="ExternalOutput")

    if debug:
        dbg = {nm: nc.dram_tensor(f"dbg_{nm}", [128, TT], F32, kind="ExternalOutput")
               for nm in ("xi", "xh", "dt", "y", "da", "u", "h")}
        dbg["xdbl"] = nc.dram_tensor("dbg_xdbl", [TT, XD], F32, kind="ExternalOutput")
    with tile.TileContext(nc) as tc:
        with (
            tc.tile_pool(name="wts", bufs=1) as wts,
            tc.tile_pool(name="mem", bufs=1) as mem,
            tc.tile_pool(name="ps", bufs=1, space="PSUM") as ps,
            tc.tile_pool(name="dram", bufs=1, space="DRAM") as dram,
        ):
            ar_in = dram.tile([TT, XD], F32, name="ar_in")
            ar_out = dram.tile([TT, XD], F32, name="ar_out", addr_space="Shared")
            bc_rows = dram.tile([2 * NST, TT], BF16, name="bc_rows")
            # ---------- small persistent weights ----------
            sb_ident = wts.tile([128, 128], F32)
            nc.sync.dma_start(out=sb_ident, in_=ident[:, :])
            sb_cw, sb_cb, sb_bdt, sb_dsk, sb_wx, sb_acol = [], [], [], [], [], []
            for d in range(NDT):
                sl = slice(d * 128, (d + 1) * 128)
                for lst, src, w in ((sb_cw, conv_w, 4), (sb_cb, conv_b, 1),
                                    (sb_bdt, b_dt, 1), (sb_dsk, d_skip, 1)):
                    t = wts.tile([128, w], F32, name=f"w{len(lst)}_{id(src) % 997}_{d}")
                    nc.sync.dma_start(out=t, in_=src[sl, :])
                    lst.append(t)
                t = wts.tile([128, XD], BF16, name=f"wx{d}")
                nc.sync.dma_start(out=t, in_=w_x[sl, :])
                sb_wx.append(t)
                t = wts.tile([128, NST], F32, name=f"ac{d}")
                nc.sync.dma_start(out=t, in_=a_cols[sl, :])
                sb_acol.append(t)
            sb_wdt = wts.tile([DTR, DLOC], BF16)
            nc.sync.dma_start(out=sb_wdt, in_=w_dt[:, :])

            # ---------- persistent activations (bf16) ----------
            sb_xh = [mem.tile([128, TT], BF16, name=f"xh{d}", tag=f"xh{d}")
                     for d in range(NDT)]
            sb_z = [mem.tile([128, TT], BF16, name=f"z{d}", tag=f"z{d}")
                    for d in range(NDT)]
            sb_dtx = [mem.tile([128, TT], BF16, name=f"dtx{d}", tag=f"dtx{d}")
                      for d in range(NDT)]
            sb_y = [mem.tile([128, TT], BF16, name=f"y{d}", tag=f"y{d}")
                    for d in range(NDT)]

            # ---------- Phase 1: in_proj (f32r) -> xi (slab), z ----------
            sb_xi = [mem.tile([128, TT], F32, name=f"xi{d}", tag="slab", bufs=7)
                     for d in range(NDT)]
            for ci, cw in enumerate(TCH):
                co = TCHO[ci]
                pts = [ps.tile([128, 512], F32, name=f"pj{ci}_{m}", tag="pj", bufs=8)
                       for m in range(8)]
                for k in range(8):
                    xsc = mem.tile([128, 512], F32R, name=f"xsc{ci}_{k}",
                                   tag="xsc", bufs=3)
                    nc.sync.dma_start(out=xsc[:, :cw],
                                      in_=xsT[k * 128:(k + 1) * 128, co:co + cw])
                    wic = mem.tile([128, 1024], F32R, name=f"wic{ci}_{k}",
                                   tag="wic", bufs=3)
                    nc.sync.dma_start(out=wic, in_=w_in[k * 128:(k + 1) * 128, :])
                    for m in range(8):
                        nc.tensor.matmul(pts[m][:, :cw],
                                         wic[:, m * 128:(m + 1) * 128],
                                         xsc[:, :cw],
                                         start=(k == 0), stop=(k == 7))
                for m in range(8):
                    if m < 4:
                        nc.scalar.copy(sb_xi[m][:, co:co + cw], pts[m][:, :cw])
                    else:
                        nc.scalar.activation(sb_z[m - 4][:, co:co + cw],
                                             pts[m][:, :cw], AF.Silu)

            # ---------- Phase 2: causal depthwise conv + silu -> xh ----------
            for d in range(NDT):
                xc = mem.tile([128, TT], F32, name=f"xc{d}", tag="slab", bufs=7)
                nc.vector.tensor_scalar_mul(xc, sb_xi[d], sb_cw[d][:, 3:4])
                for b in range(B):
                    s = b * L
                    for j in range(3):
                        o = 3 - j
                        tp = mem.tile([128, L], F32, name=f"tp{d}_{b}_{j}",
                                      tag="tap", bufs=2)
                        nc.scalar.activation(tp[:, :L - o], sb_xi[d][:, s:s + L - o],
                                             AF.Copy, scale=sb_cw[d][:, j:j + 1])
                        nc.vector.tensor_tensor(xc[:, s + o:s + L],
                                                xc[:, s + o:s + L],
                                                tp[:, :L - o], OP.add)
                nc.scalar.activation(sb_xh[d], xc, AF.Silu, bias=sb_cb[d])

            if debug:
                nc.sync.dma_start(out=dbg["xi"][:, :], in_=sb_xi[0])
                dxh = mem.tile([128, TT], F32, name="dxh", tag="slab", bufs=7)
                nc.vector.tensor_copy(dxh, sb_xh[0])
                nc.sync.dma_start(out=dbg["xh"][:, :], in_=dxh)
            # ---------- Phase 3: x_proj partials + AllReduce ----------
            for ti, (t0, twd) in enumerate(_tslices()):
                pt = ps.tile([128, XD], F32, name=f"px{ti}", tag="pj", bufs=8)
                for d in range(NDT):
                    nc.tensor.matmul(pt[:twd, :], sb_xh[d][:, t0:t0 + twd],
                                     sb_wx[d], start=(d == 0), stop=(d == NDT - 1))
                ev = mem.tile([128, XD], F32, name=f"xde{ti}", tag="xde", bufs=3)
                nc.scalar.copy(ev[:twd, :], pt[:twd, :])
                nc.sync.dma_start(out=ar_in[t0:t0 + twd, :], in_=ev[:twd, :])
            nc.gpsimd.collective_compute(
                "AllReduce", OP.add, replica_groups=[list(range(N_CORES))],
                ins=[ar_in.opt()], outs=[ar_out.opt()])

            # ---------- Phase 4: transpose x_dbl -> [96, t] bf16 ----------
            sb_xdT = mem.tile([96, TT], BF16, tag="bb", bufs=2)
            for ti, (t0, twd) in enumerate(_tslices()):
                ld = mem.tile([128, XD], F32, name=f"xl{ti}", tag="xde", bufs=3)
                nc.sync.dma_start(out=ld[:twd, :], in_=ar_out[t0:t0 + twd, :])
                pt = ps.tile([128, 128], F32, name=f"ptr{ti}", tag="pj", bufs=8)
                nc.tensor.transpose(pt[:XD, :twd], ld[:twd, :XD],
                                    sb_ident[:twd, :twd])
                nc.scalar.copy(sb_xdT[:, t0:t0 + twd], pt[:XD, :twd])
            nc.sync.dma_start(out=bc_rows[:, :], in_=sb_xdT[DTR:XD, :])

            if debug:
                nc.sync.dma_start(out=dbg["xdbl"][:, :], in_=ar_out[:, :])
            # ---------- Phase 5: dt (softplus) f32, dtx bf16 ----------
            sb_dt = [mem.tile([128, TT], F32, name=f"dt{d}", tag="slab", bufs=7)
                     for d in range(NDT)]
            for d in range(NDT):
                for ci, cw in enumerate(TCH):
                    co = TCHO[ci]
                    pt = ps.tile([128, 512], F32, name=f"pd{d}_{ci}", tag="pj",
                                 bufs=8)
                    nc.tensor.matmul(pt[:, :cw], sb_wdt[:, d * 128:(d + 1) * 128],
                                     sb_xdT[:DTR, co:co + cw], start=True, stop=True)
                    e1 = mem.tile([128, 512], F32, name=f"e{d}_{ci}", tag="sp",
                                  bufs=3)
                    nc.scalar.activation(e1[:, :cw], pt[:, :cw], AF.Exp,
                                         bias=sb_bdt[d])
                    nc.scalar.activation(sb_dt[d][:, co:co + cw], e1[:, :cw],
                                         AF.Ln, bias=1.0)
                nc.vector.tensor_tensor(sb_dtx[d], sb_dt[d], sb_xh[d], OP.mult)

            if debug:
                nc.sync.dma_start(out=dbg["dt"][:, :], in_=sb_dt[0])
            # ---------- Phase 6: selective scan ----------
            for n in range(NST):
                bb = mem.tile([128, TT], BF16, name=f"bb{n}", tag="bb", bufs=2)
                cb = mem.tile([128, TT], BF16, name=f"cb{n}", tag="cbx", bufs=2)
                for dst, row in ((bb, n), (cb, NST + n)):
                    src = bc_rows[row:row + 1, :]
                    nc.sync.dma_start(out=dst, in_=bass.AP(
                        tensor=src.tensor, offset=src.offset,
                        ap=[[0, 128]] + src.ap[1:]))
                for d in range(NDT):
                    da = mem.tile([128, TT], F32, name=f"da{n}_{d}", tag="slab",
                                  bufs=7)
                    if a_scales is not None:
                        nc.scalar.activation(da, sb_dt[d], AF.Exp,
                                             scale=float(a_scales[n]))
                    else:
                        nc.scalar.activation(da, sb_dt[d], AF.Exp,
                                             scale=sb_acol[d][:, n:n + 1])
                    nc.vector.memset(da[:, L:L + 1], 0.0)
                    u = mem.tile([128, TT], BF16, name=f"u{n}_{d}", tag="u", bufs=2)
                    nc.vector.tensor_tensor(u, sb_dtx[d], bb, OP.mult)
                    h = mem.tile([128, TT], BF16, name=f"h{n}_{d}", tag="h", bufs=2)
                    nc.vector.tensor_tensor_scan(h, da, u, 0.0, OP.mult, OP.add)
                    p = mem.tile([128, TT], BF16, name=f"p{n}_{d}", tag="p", bufs=2)
                    nc.vector.tensor_tensor(p, h, cb, OP.mult)
                    if debug and n == 0 and d == 0:
                        ddump = mem.tile([128, TT], F32, name="ddmp", tag="slab", bufs=7)
                        nc.vector.tensor_copy(ddump, da)
                        nc.sync.dma_start(out=dbg["da"][:, :], in_=ddump)
                        udump = mem.tile([128, TT], F32, name="udmp", tag="slab", bufs=7)
                        nc.vector.tensor_copy(udump, u)
                        nc.sync.dma_start(out=dbg["u"][:, :], in_=udump)
                        hdump = mem.tile([128, TT], F32, name="hdmp", tag="slab", bufs=7)
                        nc.vector.tensor_copy(hdump, h)
                        nc.sync.dma_start(out=dbg["h"][:, :], in_=hdump)
                    if n == 0:
                        nc.vector.tensor_copy(sb_y[d], p)
                    else:
                        nc.vector.tensor_tensor(sb_y[d], sb_y[d], p, OP.add)

            if debug:
                dy = mem.tile([128, TT], F32, name="dy", tag="slab", bufs=7)
                nc.vector.tensor_copy(dy, sb_y[0])
                nc.sync.dma_start(out=dbg["y"][:, :], in_=dy)
            # ---------- Phase 7: skip + gate (into z slot) ----------
            for d in range(NDT):
                sk = mem.tile([128, TT], BF16, name=f"sk{d}", tag="u", bufs=2)
                nc.vector.tensor_scalar_mul(sk, sb_xh[d], sb_dsk[d][:, 0:1])
                nc.vector.tensor_tensor(sk, sb_y[d], sk, OP.add)
                nc.vector.tensor_tensor(sb_z[d], sk, sb_z[d], OP.mult)

            # ---------- Phase 8: out_proj partials (bf16) ----------
            sb_wo = []
            for d in range(NDT):
                t = wts.tile([128, DM], BF16, name=f"wo{d}")
                nc.sync.dma_start(out=t, in_=w_out[d * 128:(d + 1) * 128, :])
                sb_wo.append(t)
            for ti, (t0, twd) in enumerate(_tslices()):
                for f in range(2):
                    pt = ps.tile([128, 512], F32, name=f"po{ti}_{f}", tag="pj",
                                 bufs=8)
                    for d in range(NDT):
                        nc.tensor.matmul(
                            pt[:twd, :], sb_z[d][:, t0:t0 + twd],
                            sb_wo[d][:, f * 512:(f + 1) * 512],
                            start=(d == 0), stop=(d == NDT - 1))
                    ev = mem.tile([128, 512], F32, name=f"oe{ti}_{f}", tag="sp",
                                  bufs=3)
                    nc.scalar.copy(ev[:twd, :], pt[:twd, :])
                    nc.sync.dma_start(out=out_p[t0:t0 + twd, f * 512:(f + 1) * 512],
                                      in_=ev[:twd, :])

    nc.compile()
    return nc


_CACHE = {}


def _get_program(a_scales_key):
    if a_scales_key not in _CACHE:
        _CACHE[a_scales_key] = build_program(
            list(a_scales_key) if a_scales_key is not None else None)
    return _CACHE[a_scales_key]


def make_inputs(x, layer_idx, emb_table, W_in, conv_w, conv_b, W_x, W_dt, b_dt,
                A_log, D_skip, W_out):
    x = np.asarray(x, np.float32)
    emb = np.asarray(emb_table, np.float32)[int(layer_idx)]
    xs = np.concatenate([np.broadcast_to(emb, (B, 1, DM)), x], axis=1)
    xsT = np.ascontiguousarray(xs.reshape(TT, DM).T)

    A = -np.exp(np.asarray(A_log, np.float64)).astype(np.float32)
    same = bool(np.all(A == A[0:1, :]))
    a_key = tuple(float(v) for v in A[0]) if same else None

    W_in = np.asarray(W_in, np.float32)
    ident = np.eye(128, dtype=np.float32)
    ins = []
    for c in range(N_CORES):
        sl = slice(c * DLOC, (c + 1) * DLOC)
        w_in_cat = np.concatenate(
            [W_in[:, c * DLOC:(c + 1) * DLOC],
             W_in[:, DI + c * DLOC:DI + (c + 1) * DLOC]], axis=1)
        ins.append({
            "xsT": xsT,
            "w_in": np.ascontiguousarray(w_in_cat),
            "conv_w": np.ascontiguousarray(np.asarray(conv_w, np.float32)[sl]),
            "conv_b": np.ascontiguousarray(
                np.asarray(conv_b, np.float32)[sl][:, None]),
            "w_x": np.ascontiguousarray(np.asarray(W_x, np.float32)[sl]).astype(ml_dtypes.bfloat16),
            "w_dt": np.ascontiguousarray(np.asarray(W_dt, np.float32)[:, sl]).astype(ml_dtypes.bfloat16),
            "b_dt": np.ascontiguousarray(
                np.asarray(b_dt, np.float32)[sl][:, None]),
            "a_cols": np.ascontiguousarray(A[sl]),
            "d_skip": np.ascontiguousarray(
                np.asarray(D_skip, np.float32)[sl][:, None]),
            "w_out": np.ascontiguousarray(np.asarray(W_out, np.float32)[sl]).astype(ml_dtypes.bfloat16),
            "ident": ident,
        })
    return ins, a_key


def kernel(**inputs) -> np.ndarray:
    ins, a_key = make_inputs(**inputs)
    nc = _get_program(a_key)
    res = run_bass_kernel_spmd(nc, ins, core_ids=list(range(N_CORES)))
    out = np.zeros((TT, DM), np.float64)
    for c in range(N_CORES):
        out += res.results[c]["out_p"]
    return out.astype(np.float32).reshape(B, L, DM)
